# revision 1
# baseline (speedup 1.0000x reference)
"""Trainium2 Bass kernel for nn_MultiHeadAttention (N=2048, D=1024, H=16, causal).

Sharding: 16 heads split across 8 NeuronCores (2 heads/core, tensor-parallel
on the head dim).  Each core projects Q^T/K^T (its 128 head-dims x full
sequence) and V for its heads, computes causal attention in scores-transposed
layout ([nk, nq] blocks, softmax along the nk partition axis, denominator via
a ones-column appended to V), applies its 128-row slice of Wo, and writes an
fp16 partial [2048, 1024] output.  The host sums the 8 partials and adds bo
("all-reduce after W_o" done host-side).

Optimizations vs the f32r baseline (133.2us -> 77.1us modeled):
  - Q/K path in fp8e4m3 (inputs plus 16x-prescaled weights; the 256x score
    factor folds into the exp scale) with DoubleRow projection matmuls
    (256-wide contraction at 0.5 cycles/row).  V path, probs, attnT and Wo
    in fp16.  Measured end-to-end rel err 1.4e-2 vs the 2e-2 gate.
  - Inputs streamed as 512-column tiles ([128, 8, 512] packed host-side) as
    per-tile (q, k, v) triplets on one DMA queue, so tile pipelines unlock
    progressively; output staged in fp16 per 128-row band.
  - Causally trimmed score blocks (free widths 512/384/256/128 on the
    diagonal wedge) and strip-decomposed PV accumulation keep attention PE
    work near the causal minimum.
  - The PE SEQ is in-order and sem-waits block it, so emission interleaves
    independent "filler" work (V projection, previous tiles' PV / normalize /
    Wo) between score groups, paced against the Activation-engine exp train.
  - Softmax denominators kept as fp16 at 1/16 scale (range headroom); the
    1/16 compensation folds into the output staging copies.  A few warmup
    matmuls hold the PE p-state at full clock through the DMA front.
"""
import os
import sys

for _p in ("/opt/trn_rl_repo", "/root/.axon_site/_ro/trn_rl_repo"):
    if os.path.isdir(_p) and _p not in sys.path:
        sys.path.append(_p)

import numpy as np

import concourse.bass as bass
import concourse.mybir as mybir
from concourse import bacc
from concourse.bass_utils import run_bass_kernel_spmd
from concourse.tile import TileContext
from contextlib import ExitStack

N = 2048
D = 1024
NCORES = 8
DL = 128

F32 = mybir.dt.float32
F16 = mybir.dt.float16
BF = mybir.dt.bfloat16
F8 = mybir.dt.float8e4

# fp8 Q/K path: q, k, Wq, Wk in e4m3 (weights host-scaled x16 to clear the
# e4m3 denormal floor; bq, bk scaled to match; the extra 16*16 factor on the
# scores folds into the exp scale).  Softmax shift-invariance plus the flat
# attention profile makes the added error ~0.3%.  V path and Wo stay bf16.
QK_FP8 = True
EXP_SCALE = 0.125 / (256.0 if QK_FP8 else 1.0)


def build_nc(opts=None):
    qk_dt = F8 if QK_FP8 else BF
    nc = bacc.Bacc("TRN2", target_bir_lowering=False, debug=False,
                   num_devices=NCORES)

    qP = nc.dram_tensor("qP", [128, 8, N], qk_dt, kind="ExternalInput")
    kP = nc.dram_tensor("kP", [128, 8, N], qk_dt, kind="ExternalInput")
    vP = nc.dram_tensor("vP", [128, 8, N], F16, kind="ExternalInput")
    wqP = nc.dram_tensor("wqP", [128, 8, DL], qk_dt, kind="ExternalInput")
    wkP = nc.dram_tensor("wkP", [128, 8, DL], qk_dt, kind="ExternalInput")
    wvP = nc.dram_tensor("wvP", [128, 8, DL], F16, kind="ExternalInput")
    woP = nc.dram_tensor("woP", [DL, D], F16, kind="ExternalInput")
    bqk = nc.dram_tensor("bqk", [DL, 2], F32, kind="ExternalInput")
    bvb = nc.dram_tensor("bvb", [DL, DL], F32, kind="ExternalInput")
    out = nc.dram_tensor("out", [N, D], F16, kind="ExternalOutput")

    AF = mybir.ActivationFunctionType
    ALU = mybir.AluOpType

    with TileContext(nc) as tc, ExitStack() as ctx:
        const = ctx.enter_context(tc.tile_pool(name="const", bufs=1))
        big = ctx.enter_context(tc.tile_pool(name="big", bufs=1))
        colp = ctx.enter_context(tc.tile_pool(name="colp", bufs=1))
        probsp = ctx.enter_context(tc.tile_pool(name="probsp", bufs=1))
        dnp = ctx.enter_context(tc.tile_pool(name="dnp", bufs=2))
        rcp = ctx.enter_context(tc.tile_pool(name="rcp", bufs=2))
        outp = ctx.enter_context(tc.tile_pool(name="outp", bufs=4))

        # ---- constants + input streaming, all on the sync queue in the
        # order transfers should hit the DMA engines: per-tile triplets
        # (q_c, k_c, v_c) so tile c's full pipeline unlocks ~9us apart ----
        wq = const.tile([128, 8, DL], qk_dt)
        nc.sync.dma_start(wq[:], wqP[:])
        bqk_t = const.tile([DL, 2], F32)
        nc.sync.dma_start(bqk_t[:], bqk[:])
        qc, kc, vc = [], [], []

        def load_col(lst, name, dram, c, dt=F16):
            t = colp.tile([128, 8, 512], dt, name=f"{name}{c}")
            nc.sync.dma_start(t[:], dram[:, :, 512 * c:512 * (c + 1)])
            lst.append(t)

        load_col(qc, "qc", qP, 0, qk_dt)
        wk = const.tile([128, 8, DL], qk_dt)
        nc.sync.dma_start(wk[:], wkP[:])
        load_col(kc, "kc", kP, 0, qk_dt)
        load_col(qc, "qc", qP, 1, qk_dt)
        load_col(kc, "kc", kP, 1, qk_dt)
        wv = const.tile([128, 8, DL], F16)
        nc.sync.dma_start(wv[:], wvP[:])
        bvb_t = const.tile([DL, DL], F32)
        nc.sync.dma_start(bvb_t[:], bvb[:])
        load_col(vc, "vc", vP, 0)
        load_col(qc, "qc", qP, 2, qk_dt)
        load_col(kc, "kc", kP, 2, qk_dt)
        load_col(vc, "vc", vP, 1)
        wo = const.tile([DL, D], F16)
        nc.sync.dma_start(wo[:], woP[:])
        load_col(qc, "qc", qP, 3, qk_dt)
        load_col(kc, "kc", kP, 3, qk_dt)
        load_col(vc, "vc", vP, 2)
        load_col(vc, "vc", vP, 3)

        ones64 = const.tile([1, 64], F16)
        nc.vector.memset(ones64[:], 1.0)
        ones512 = const.tile([1, 512], F16)
        nc.vector.memset(ones512[:], 1.0)

        # ---- persistent activations ----
        QT = big.tile([128, N], F16)
        KT = big.tile([128, N], F16)
        Vaug0 = big.tile([128, 16, 65], F16)
        Vaug1 = big.tile([128, 16, 65], F16)
        nc.vector.memset(Vaug0[:, :, 64:65], 1.0)
        nc.vector.memset(Vaug1[:, :, 64:65], 1.0)
        attnT = big.tile([128, N], F16)

        # probs bookkeeping: probs_ref[(t, h)] = (pairs, pw1, pw2)
        # probs bookkeeping: probs_ref[(t, h)] = (pair_probs, pw1, pw2)
        probs_ref = {}

        with tc.tile_pool(name="scp", bufs=3, space="PSUM") as scp, \
             tc.tile_pool(name="wkp", bufs=3, space="PSUM") as wkp, \
             tc.tile_pool(name="pvp", bufs=2, space="PSUM") as pvp:

            # The PE SEQ is in-order and an instruction's sem-waits block it,
            # so a PSUM-slot allocation or Ldweights that waits on a consumer/
            # producer stalls ALL later PE work.  Emission is therefore a
            # fine-grained software-pipelined round-robin: "filler" groups
            # whose deps resolved a while ago (V projection, earlier PV
            # strips / normalize / Wo) are emitted between score-tile groups.
            fillers = []

            probs_ref = {}

            def emit_score_block(t, h, b):
                # single nk block; wedge blocks (b >= 4t) causally trimmed
                r0, r1 = 64 * h, 64 * (h + 1)
                wdg = b - 4 * t
                off = 128 * wdg if wdg >= 0 else 0
                width = 512 - off
                sc = scp.tile([128, 512], F32, name="sc")
                nc.tensor.matmul(
                    sc[:, 0:width],
                    KT[r0:r1, 128 * b:128 * (b + 1)],
                    QT[r0:r1, 512 * t + off:512 * (t + 1)],
                    start=True, stop=True, tile_position=(64 * h, 0))
                pb = probsp.tile([128, width], F16, name=f"pb{t}{h}{b}")
                nc.scalar.activation(pb[:], sc[:, 0:width], AF.Exp,
                                     scale=EXP_SCALE)
                if wdg >= 0:
                    nc.gpsimd.affine_select(
                        out=pb[:, 0:128], in_=pb[:, 0:128],
                        compare_op=ALU.is_ge, fill=0.0,
                        base=0, pattern=[[1, 128]], channel_multiplier=-1)
                probs_ref[(t, h)][b] = pb

            def probs_slice(t, h, b, s):
                pb = probs_ref[(t, h)][b]
                wdg = max(0, b - 4 * t)
                c0 = 128 * (s - wdg)
                return pb[:, c0:c0 + 128]

            def emit_vproj(t, bb):
                b = 4 * t + bb
                vp = wkp.tile([128, 512], F32, name="wk")
                for j in range(8):
                    nc.tensor.matmul(
                        vp[:, 0:128],
                        vc[t][:, j, 128 * bb:128 * (bb + 1)],
                        wv[:, j, :], start=(j == 0), stop=(j == 7))
                nc.vector.scalar_tensor_tensor(
                    Vaug0[:, b, 0:64], vp[:, 0:64], 1.0, bvb_t[:, 0:64],
                    ALU.mult, ALU.add)
                nc.vector.scalar_tensor_tensor(
                    Vaug1[:, b, 0:64], vp[:, 64:128], 1.0, bvb_t[:, 64:128],
                    ALU.mult, ALU.add)

            pvhs = {}

            def emit_pv(t, h, s):
                if s == 0:
                    pvhs[(t, h)] = pvp.tile([128, 512], F32, name="pv")
                pvh = pvhs[(t, h)]
                last = 4 * t + s
                for b in range(last + 1):
                    nc.tensor.matmul(
                        pvh[0:65, 128 * s:128 * (s + 1)],
                        Vaug0[:, b, :] if h == 0 else Vaug1[:, b, :],
                        probs_slice(t, h, b, s),
                        start=(b == 0), stop=(b == last))

            def emit_norm(t, h):
                # denom row -> SBUF, PE-broadcast to 64 partitions, recip to
                # SBUF, then scale (DVE ops may read at most one PSUM input)
                pvh = pvhs[(t, h)]
                dn = dnp.tile([1, 512], F16, name="dn")
                nc.vector.tensor_scalar_mul(dn[:], pvh[64:65, :], 1.0 / 16.0)
                bcp = wkp.tile([64, 512], F32, name="wk")
                nc.tensor.matmul(bcp[:], ones64[:], dn[:],
                                 start=True, stop=True)
                rcs = rcp.tile([64, 512], F32, name="rc")
                nc.vector.reciprocal(rcs[:], bcp[:])
                nc.vector.tensor_mul(
                    attnT[64 * h:64 * (h + 1), 512 * t:512 * (t + 1)],
                    pvh[0:64, :], rcs[:])

            def emit_wo(t, m):
                ob = outp.tile([128, 1024], F16, name="ob")
                for u in range(2):
                    wps = wkp.tile([128, 512], F32, name="wk")
                    nc.tensor.matmul(wps[:],
                                     attnT[:, 128 * m:128 * (m + 1)],
                                     wo[:, 512 * u:512 * (u + 1)],
                                     start=True, stop=True)
                    # attnT carries a 16x factor (fp16 denominator headroom);
                    # fold the 1/16 into the staging copy.  The last tile's
                    # copies go to the otherwise-idle Act engine to shorten
                    # the tail chain.
                    if m >= 12 and u == 1:
                        nc.scalar.activation(ob[:, 512 * u:512 * (u + 1)],
                                             wps[:], AF.Copy,
                                             scale=1.0 / 16.0)
                    else:
                        nc.vector.tensor_scalar_mul(
                            ob[:, 512 * u:512 * (u + 1)], wps[:], 1.0 / 16.0)
                nc.sync.dma_start(out[128 * m:128 * (m + 1), :], ob[:])

            def emit_proj(src_c, w, bcol, dst, t):
                ps = scp.tile([128, 512], F32, name="sc")
                if QK_FP8:
                    # fp8 DoubleRow: contract 256 per step (2 packed rows),
                    # 0.5 cycles/row -- w[:, 2jj:2jj+2, :] and the matching
                    # input slice share the (partition, pair) -> dmodel map
                    for jj in range(4):
                        nc.tensor.matmul(
                            ps[:], w[:, 2 * jj:2 * jj + 2, :],
                            src_c[:, 2 * jj:2 * jj + 2, :],
                            start=(jj == 0), stop=(jj == 3),
                            perf_mode=mybir.MatmulPerfMode.DoubleRow)
                else:
                    for j in range(8):
                        nc.tensor.matmul(ps[:], w[:, j, :], src_c[:, j, :],
                                         start=(j == 0), stop=(j == 7))
                nc.vector.tensor_scalar_add(
                    dst[:, 512 * t:512 * (t + 1)], ps[:],
                    bqk_t[:, bcol:bcol + 1])

            # PE warmup: keep the tensor engine continuously busy through
            # the initial DMA front so the p-state ramp completes before the
            # first projection (ramp resets on idle; full clock after 3us).
            for _ in range(4):
                wu = wkp.tile([128, 512], F32, name="wk")
                nc.tensor.matmul(wu[0:64, :], ones64[:], ones512[:],
                                 start=True, stop=True)

            groups_left = sum(4 * t + 4 for t in range(4)) * 2

            def pop_fillers():
                import math
                k = max(2, min(4, math.ceil(len(fillers) / max(1, groups_left))))
                for _ in range(k):
                    if fillers:
                        fillers.pop(0)()

            def queue_pipe(t):
                fillers.extend(lambda bb=bb, c=t: emit_vproj(c, bb)
                               for bb in range(4))
                fillers.extend(lambda s=s, t=t: emit_pv(t, 0, s)
                               for s in range(4))
                pipe = [lambda t=t: emit_pv(t, 1, 0),
                        lambda t=t: emit_pv(t, 1, 1),
                        lambda t=t: emit_norm(t, 0),
                        lambda t=t: emit_pv(t, 1, 2),
                        lambda t=t: emit_pv(t, 1, 3),
                        lambda t=t: emit_norm(t, 1)]
                pipe.extend(lambda m=4 * t + s, t=t: emit_wo(t, m)
                            for s in range(4))
                fillers.extend(pipe)

            for t in range(4):
                emit_proj(qc[t], wq, 0, QT, t)
                if t == 0:
                    for _ in range(2):
                        wu = wkp.tile([128, 512], F32, name="wk")
                        nc.tensor.matmul(wu[0:64, :], ones64[:], ones512[:],
                                         start=True, stop=True)
                emit_proj(kc[t], wk, 1, KT, t)
                # pipes run one slot late: tile t-1's V projection + PV +
                # norm + Wo ride the bubbles of tile t's exp train (v-columns
                # are deferred behind q/k in the DMA order)
                if t >= 1:
                    queue_pipe(t - 1)
                for h in range(2):
                    probs_ref[(t, h)] = {}
                    for b in range(4 * t + 4):
                        emit_score_block(t, h, b)
                        groups_left -= 1
                        pop_fillers()
            queue_pipe(3)
            while fillers:
                fillers.pop(0)()

    nc.compile()
    return nc


def make_in_maps(q, k, v, Wq, bq, Wk, bk, Wv, bv, Wo, bo):
    import ml_dtypes
    bf16 = ml_dtypes.bfloat16
    fp8 = ml_dtypes.float8_e4m3
    f32 = np.float32
    qk_np = fp8 if QK_FP8 else np.float16
    wscale = 16.0 if QK_FP8 else 1.0

    def pack_cols(x, dt):
        # [N, D] input -> x.T [D, N] -> [128, 8, N] with row (j*128+p) at
        # [p, j, :]
        xt = np.ascontiguousarray(x.T.astype(f32))
        return np.ascontiguousarray(
            xt.reshape(8, 128, N).transpose(1, 0, 2)).astype(dt)

    qPa, kPa = pack_cols(q, qk_np), pack_cols(k, qk_np)
    vPa = pack_cols(v, np.float16)
    WqT = Wq.T.astype(f32) * wscale
    WkT = Wk.T.astype(f32) * wscale
    WvT = Wv.T.astype(f32)
    WoT = Wo.T.astype(f32)

    def pack_w(WT, c, dt):
        # [D, DL] column slice -> [128, 8, DL]
        sl = np.ascontiguousarray(WT[:, DL * c:DL * (c + 1)])
        return np.ascontiguousarray(
            sl.reshape(8, 128, DL).transpose(1, 0, 2)).astype(dt)

    in_maps = []
    for c in range(NCORES):
        d0 = DL * c
        in_maps.append({
            "qP": qPa, "kP": kPa, "vP": vPa,
            "wqP": pack_w(WqT, c, qk_np),
            "wkP": pack_w(WkT, c, qk_np),
            "wvP": pack_w(WvT, c, np.float16),
            "woP": np.ascontiguousarray(WoT[d0:d0 + DL, :]).astype(np.float16),
            "bqk": np.ascontiguousarray(
                np.stack([bq[d0:d0 + DL] * wscale,
                          bk[d0:d0 + DL] * wscale], axis=1)).astype(f32),
            "bvb": np.ascontiguousarray(
                np.tile(bv[d0:d0 + DL][None, :], (DL, 1))).astype(f32),
        })
    return in_maps


_NC_CACHE = None


def _get_nc():
    global _NC_CACHE
    if _NC_CACHE is None:
        _NC_CACHE = build_nc()
    return _NC_CACHE


def kernel(q, k, v, Wq, bq, Wk, bk, Wv, bv, Wo, bo):
    """Full-input / full-output entry point (harness contract)."""
    q, k, v = np.asarray(q), np.asarray(k), np.asarray(v)
    Wq, bq, Wk, bk = np.asarray(Wq), np.asarray(bq), np.asarray(Wk), np.asarray(bk)
    Wv, bv, Wo, bo = np.asarray(Wv), np.asarray(bv), np.asarray(Wo), np.asarray(bo)
    nc = _get_nc()
    in_maps = make_in_maps(q, k, v, Wq, bq, Wk, bk, Wv, bv, Wo, bo)
    res = run_bass_kernel_spmd(nc, in_maps, list(range(NCORES)))
    acc = res.results[0]["out"].astype(np.float64)
    for c in range(1, NCORES):
        acc += res.results[c]["out"]
    acc += bo.astype(np.float64)
    return acc.astype(np.float32)

